# revision 12
# baseline (speedup 1.0000x reference)
"""Trainium2 Bass kernel for nn_DSModelMultiQ (Dempster-Shafer rule model).

Pipeline (per batch sample):
  xg = X[:, lit_feat_idx]                      gather      [B, L]
  truth = op-dependent compare(xg, lit_value)  elementwise [B, L]
  fired = (truth @ lit2rule >= rule_len - .5)  -> computed as a product of the
          3 gathered truth rows of each rule (exact: every rule is a
          conjunction of exactly 3 literals, duplicates just repeat a factor)
  masses = softmax(rule_mass_params)           [R, K+1]
  q/w = exp(fired @ [log(m_k+om+eps) | log(om+eps)])
  out  = (relu(q-w) + w*prior) / max(sum(relu(q-w)) + w, eps)

Sharding: data-parallel over batch B across 8 NeuronCores (B=8192 -> 1024/core).
Each core holds the full rule base. Pure SPMD, no collectives; host only
shards X, extracts per-rule literal ids from lit2rule (index bookkeeping), and
rearranges metadata into per-partition-scalar layouts.

Device layout choices:
  - truth^T [L, B_local] with L on partitions: per-literal value/op constants
    become per-partition scalars for tensor_scalar ops; staged to DRAM so the
    per-rule literal rows can be row-gathered by indirect DMA.
  - fired^T [R-chunk, B_local] = g0*g1*g2 of the gathered rows feeds the
    class-mass matmul directly as the stationary operand.
  - the class-mass matmul uses a split-bf16 (hi+lo) log-mass operand for
    fp32-level accuracy at bf16 PE throughput; accumulated across all 64 rule
    chunks in packed PSUM banks (memset + flags=0 accumulate).
"""

import numpy as np
import ml_dtypes  # noqa: F401  (bf16 dtype availability)

from concourse import bacc
import concourse.bass as bass
import concourse.mybir as mybir
import concourse.tile as tile
from concourse.bass_utils import run_bass_kernel_spmd

F32 = mybir.dt.float32
BF16 = mybir.dt.bfloat16
I32 = mybir.dt.int32
AF = mybir.ActivationFunctionType
OP = mybir.AluOpType
AX = mybir.AxisListType

EPS = 1e-12

# full problem dims
B, F, L, R, K = 8192, 128, 4096, 8192, 64
N_CORES = 8


def build_nc2(BL, L_, R_, K_, nrep=1, chunk_kinds=None):
    """Per-core Bass program (gather-based fired). All 8 cores run this same
    program on different input data (pure SPMD)."""
    LC = L_ // 128
    RC = R_ // 128
    KP = K_ + 1
    W2 = 2 * KP
    NBC = BL // 128
    if chunk_kinds is None:
        chunk_kinds = [-1] * LC

    nc = bacc.Bacc(None, target_bir_lowering=False)

    xT = nc.dram_tensor("xT", [F, BL], F32, kind="ExternalInput")
    fidx = nc.dram_tensor("fidx", [128, LC], I32, kind="ExternalInput")
    lv = nc.dram_tensor("lv", [128, LC], F32, kind="ExternalInput")
    ca = nc.dram_tensor("ca", [128, LC], F32, kind="ExternalInput")
    cb = nc.dram_tensor("cb", [128, LC], F32, kind="ExternalInput")
    cc = nc.dram_tensor("cc", [128, LC], F32, kind="ExternalInput")
    idx0 = nc.dram_tensor("idx0", [128, RC], I32, kind="ExternalInput")
    idx1 = nc.dram_tensor("idx1", [128, RC], I32, kind="ExternalInput")
    idx2 = nc.dram_tensor("idx2", [128, RC], I32, kind="ExternalInput")
    rmp = nc.dram_tensor("rmp", [R_, KP], F32, kind="ExternalInput")
    prior = nc.dram_tensor("prior", [128, K_], F32, kind="ExternalInput")
    out = nc.dram_tensor("out", [BL, K_], F32, kind="ExternalOutput")

    with tile.TileContext(nc) as tc:
        with (
            tc.tile_pool(name="consts", bufs=1) as cp,
            tc.tile_pool(name="persist", bufs=1) as pp,
            tc.tile_pool(name="dramp", bufs=1, space="DRAM") as dp,
            tc.tile_pool(name="prep", bufs=3) as prp,
            tc.tile_pool(name="xgp", bufs=3) as xgp,
            tc.tile_pool(name="tmp", bufs=3) as tp,
            tc.tile_pool(name="gp", bufs=4) as gpl,
            tc.tile_pool(name="firedp", bufs=2) as fpool,
            tc.tile_pool(name="psum2", bufs=1, space="PSUM") as p2,
            tc.tile_pool(name="ep", bufs=2) as ep,
        ):
            fidx_sb = cp.tile([128, LC], I32)
            nc.scalar.dma_start(fidx_sb[:], fidx.ap())
            lv_sb = cp.tile([128, LC], F32)
            nc.scalar.dma_start(lv_sb[:], lv.ap())
            ca_sb = cp.tile([128, LC], F32)
            nc.scalar.dma_start(ca_sb[:], ca.ap())
            cb_sb = cp.tile([128, LC], F32)
            nc.scalar.dma_start(cb_sb[:], cb.ap())
            cc_sb = cp.tile([128, LC], F32)
            nc.scalar.dma_start(cc_sb[:], cc.ap())
            idx_sb = []
            for j, h in enumerate((idx0, idx1, idx2)):
                t = cp.tile([128, RC], I32, name=f"idx_sb{j}")
                nc.scalar.dma_start(t[:], h.ap())
                idx_sb.append(t)
            prior_sb = cp.tile([128, K_], F32)
            nc.scalar.dma_start(prior_sb[:], prior.ap())
            epsb = cp.tile([128, 1], F32)
            nc.vector.memset(epsb[:], EPS)

            for _rep in range(nrep):
                # prep: per-rule log-mass split (hi|lo bf16)
                logsplit = pp.tile([128, RC * W2], BF16)
                for rc in range(RC):
                    rmp_sb = prp.tile([128, KP], F32)
                    nc.scalar.dma_start(rmp_sb[:], rmp.ap()[rc * 128:(rc + 1) * 128, :])
                    negmx = prp.tile([128, 1], F32)
                    nc.vector.tensor_reduce(negmx[:], rmp_sb[:], AX.X, OP.max, negate=True)
                    e = prp.tile([128, KP], F32)
                    zs = prp.tile([128, 1], F32)
                    nc.scalar.activation(e[:], rmp_sb[:], AF.Exp, bias=negmx[:, 0:1],
                                         accum_out=zs[:, 0:1])
                    rz = prp.tile([128, 1], F32)
                    nc.vector.reciprocal(rz[:], zs[:])
                    s = prp.tile([128, K_], F32)
                    nc.vector.tensor_scalar(s[:], e[:, 0:K_], e[:, K_:KP], None, OP.add)
                    logfull = prp.tile([128, KP], F32)
                    nc.scalar.activation(logfull[:, 0:K_], s[:], AF.Ln,
                                         bias=epsb[:, 0:1], scale=rz[:, 0:1])
                    nc.scalar.activation(logfull[:, K_:KP], e[:, K_:KP], AF.Ln,
                                         bias=epsb[:, 0:1], scale=rz[:, 0:1])
                    hi = logsplit[:, rc * W2: rc * W2 + KP]
                    lo = logsplit[:, rc * W2 + KP: (rc + 1) * W2]
                    nc.vector.tensor_copy(hi, logfull[:])
                    nc.vector.tensor_tensor(lo, logfull[:], hi, OP.subtract)

                # truth^T computed per chunk then staged to DRAM for row-gather
                truth_dram = dp.tile([L_, BL], BF16)
                CMP = {0: OP.is_equal, 1: OP.is_lt, 2: OP.is_gt}
                for lc in range(LC):
                    xg = xgp.tile([128, BL], F32)
                    nc.gpsimd.indirect_dma_start(
                        out=xg[:], out_offset=None,
                        in_=xT.ap(),
                        in_offset=bass.IndirectOffsetOnAxis(ap=fidx_sb[:, lc:lc + 1], axis=0),
                    )
                    if chunk_kinds[lc] in CMP:
                        # pure-op chunk (host sorts literals by op): one compare
                        truth_sb = tp.tile([128, BL], BF16)
                        nc.vector.tensor_scalar(truth_sb[:], xg[:],
                                                lv_sb[:, lc:lc + 1], None,
                                                CMP[chunk_kinds[lc]])
                        nc.sync.dma_start(truth_dram[lc * 128:(lc + 1) * 128, :],
                                          truth_sb[:])
                        continue
                    # truth = a + b*(xg<=v) + c*(xg<v)  with per-literal a,b,c
                    t1 = tp.tile([128, BL], BF16)
                    nc.vector.tensor_scalar(t1[:], xg[:], lv_sb[:, lc:lc + 1],
                                            cb_sb[:, lc:lc + 1], OP.is_le, op1=OP.mult)
                    t2 = tp.tile([128, BL], BF16)
                    nc.vector.tensor_scalar(t2[:], xg[:], lv_sb[:, lc:lc + 1],
                                            cc_sb[:, lc:lc + 1], OP.is_lt, op1=OP.mult)
                    t12 = tp.tile([128, BL], BF16)
                    nc.vector.tensor_tensor(t12[:], t1[:], t2[:], OP.add)
                    truth_sb = tp.tile([128, BL], BF16)
                    nc.scalar.activation(truth_sb[:], t12[:],
                                         AF.Identity, bias=ca_sb[:, lc:lc + 1])
                    nc.sync.dma_start(truth_dram[lc * 128:(lc + 1) * 128, :], truth_sb[:])

                # mass-matmul accumulators: NBC slots of width W2 packed
                # 3-per-PSUM-bank; memset data once, then always flags=0
                # matmuls (overwrite-or-accumulate is correct either way).
                nbank = (NBC + 2) // 3
                p2t = []
                for bnk in range(nbank):
                    nslot = min(3, NBC - 3 * bnk)
                    t = p2.tile([128, nslot * W2], F32, name=f"p2_{bnk}")
                    nc.vector.memset(t[:], 0.0)
                    p2t.append(t)

                def p2slice(bc):
                    bnk, sl = divmod(bc, 3)
                    return p2t[bnk][:, sl * W2:(sl + 1) * W2]

                # fired^T per rule chunk = product of 3 gathered truth rows
                for rc in range(RC):
                    gs = []
                    for j in range(3):
                        g = gpl.tile([128, BL], BF16, name=f"g{j}")
                        nc.gpsimd.indirect_dma_start(
                            out=g[:], out_offset=None,
                            in_=truth_dram[:],
                            in_offset=bass.IndirectOffsetOnAxis(
                                ap=idx_sb[j][:, rc:rc + 1], axis=0),
                        )
                        gs.append(g)
                    g01 = tp.tile([128, BL], BF16)
                    nc.vector.tensor_tensor(g01[:], gs[0][:], gs[1][:], OP.mult)
                    firedT = fpool.tile([128, BL], BF16)
                    nc.vector.tensor_tensor(firedT[:], g01[:], gs[2][:], OP.mult)
                    for bc in range(NBC):
                        nc.tensor.matmul(
                            p2slice(bc),
                            lhsT=firedT[:, bc * 128:(bc + 1) * 128],
                            rhs=logsplit[:, rc * W2:(rc + 1) * W2],
                            start=False, stop=(rc == RC - 1),
                            skip_group_check=True,
                        )

                # epilogue per output row chunk
                for bc in range(NBC):
                    sall = ep.tile([128, W2], F32)
                    nc.vector.tensor_copy(sall[:], p2slice(bc))
                    logq = ep.tile([128, KP], F32)
                    nc.vector.tensor_tensor(logq[:], sall[:, 0:KP], sall[:, KP:W2], OP.add)
                    qw = ep.tile([128, KP], F32)
                    nc.scalar.activation(qw[:], logq[:], AF.Exp)
                    negw = ep.tile([128, 1], F32)
                    nc.vector.tensor_scalar(negw[:], qw[:, K_:KP], -1.0, None, OP.mult)
                    belief = ep.tile([128, K_], F32)
                    bsum = ep.tile([128, 1], F32)
                    nc.scalar.activation(belief[:], qw[:, 0:K_], AF.Relu,
                                         bias=negw[:, 0:1], accum_out=bsum[:, 0:1])
                    total = ep.tile([128, 1], F32)
                    nc.vector.tensor_scalar(total[:], bsum[:], qw[:, K_:KP], EPS,
                                            OP.add, op1=OP.max)
                    rtot = ep.tile([128, 1], F32)
                    nc.vector.reciprocal(rtot[:], total[:])
                    wp = ep.tile([128, K_], F32)
                    nc.vector.tensor_scalar(wp[:], prior_sb[:], qw[:, K_:KP], None, OP.mult)
                    num = ep.tile([128, K_], F32)
                    nc.vector.tensor_tensor(num[:], belief[:], wp[:], OP.add)
                    outt = ep.tile([128, K_], F32)
                    nc.vector.tensor_scalar(outt[:], num[:], rtot[:, 0:1], None, OP.mult)
                    nc.sync.dma_start(out.ap()[bc * 128:(bc + 1) * 128, :], outt[:])

    return nc


# kept for reference/AB-testing by sim_test.py (the GEMM formulation, ~1.2ms HW)
def build_nc(BL, L_, R_, K_, nrep=1):
    raise NotImplementedError("GEMM variant removed; see git-less history in transcripts")


def host_prep(X, lit_value, lit2rule, rule_len, rule_mass_params, prior,
              lit_feat_idx, lit_op_code, BL, L_, R_, K_, n_cores):
    """Pure data-marshaling on host: shard X over batch, extract each rule's
    3 literal ids from the lit2rule incidence matrix (index bookkeeping),
    rearrange per-literal metadata into [128, chunks] per-partition-scalar
    layout."""
    X = np.asarray(X, dtype=np.float32)
    lit_value = np.asarray(lit_value, dtype=np.float32)
    lit2rule = np.asarray(lit2rule, dtype=np.float32)
    rule_mass_params = np.asarray(rule_mass_params, dtype=np.float32)
    prior = np.asarray(prior, dtype=np.float32)
    op = np.asarray(lit_op_code)

    # each rule has exactly 3 literal slots (duplicates appear as counts 2/3)
    lT = lit2rule.T
    r_idx, l_idx = np.nonzero(lT)
    cnt = lT[r_idx, l_idx].astype(np.int64)
    rl = np.repeat(l_idx, cnt)
    assert rl.size == 3 * R_, rl.size
    rule_lits = rl.reshape(R_, 3).astype(np.int32)
    # sort literals by op class so most 128-chunks need a single compare op
    perm = np.argsort(op, kind="stable").astype(np.int32)
    inv = np.empty_like(perm)
    inv[perm] = np.arange(L_, dtype=np.int32)
    lit_value = lit_value[perm]
    lit_feat_idx = np.asarray(lit_feat_idx)[perm]
    op = np.asarray(op)[perm]
    rule_lits = inv[rule_lits]
    ops_c = op.reshape(-1, 128)
    chunk_kinds = [int(c[0]) if (c == c[0]).all() else -1 for c in ops_c]

    def col128(v):
        return np.ascontiguousarray(np.asarray(v).reshape(-1, 128).T)

    fidx_r = col128(np.asarray(lit_feat_idx, dtype=np.int32))
    lv_r = col128(lit_value)
    # truth = a + b*(xg<=v) + c*(xg<v);  op0 '==': le-lt, op1 '<': lt, op2 '>': 1-le
    a = (op == 2).astype(np.float32)
    b = ((op == 0).astype(np.float32) - (op == 2).astype(np.float32))
    c = ((op == 1).astype(np.float32) - (op == 0).astype(np.float32))
    ca_r, cb_r, cc_r = col128(a), col128(b), col128(c)
    prior_r = np.ascontiguousarray(np.broadcast_to(prior.reshape(1, K_), (128, K_)))

    shared = {
        "fidx": fidx_r, "lv": lv_r, "ca": ca_r, "cb": cb_r, "cc": cc_r,
        "rmp": np.ascontiguousarray(rule_mass_params), "prior": prior_r,
    }
    for j in range(3):
        shared[f"idx{j}"] = col128(rule_lits[:, j])
    in_maps = []
    for cid in range(n_cores):
        m = dict(shared)
        m["xT"] = np.ascontiguousarray(X[cid * BL:(cid + 1) * BL, :].T)
        in_maps.append(m)
    return in_maps, chunk_kinds


_NC_CACHE = {}


def kernel(**inputs) -> np.ndarray:
    BL = B // N_CORES
    in_maps, chunk_kinds = host_prep(
        inputs["X"], inputs["lit_value"], inputs["lit2rule"], inputs["rule_len"],
        inputs["rule_mass_params"], inputs["prior"], inputs["lit_feat_idx"],
        inputs["lit_op_code"], BL, L, R, K, N_CORES,
    )
    key = (BL, L, R, K, tuple(chunk_kinds))
    if key not in _NC_CACHE:
        nc = build_nc2(BL, L, R, K, chunk_kinds=chunk_kinds)
        nc.finalize()
        _NC_CACHE[key] = nc
    nc = _NC_CACHE[key]
    res = run_bass_kernel_spmd(nc, in_maps, core_ids=list(range(N_CORES)))
    return np.concatenate([r["out"] for r in res.results], axis=0)
